# revision 8
# baseline (speedup 1.0000x reference)
"""Trainium2 Bass kernel for BinaryRelativePositionEmbedding.

Math: out[b,h,l,m] = q[b,h,l,:] . rp[m,:],  rp = bits @ emb, where
bits[m,:] are the 12 two's-complement bits of position (m - L + 1).

Key identity: out[l, m] = sum_b bits[m,b] * s[l,b] with s = q @ emb^T
(rank 12).  The pattern v(m) = (m - (L-1)) & 4095 ranges over all 12-bit
values except 2048, so each row-tile of the output is a subset-sum table
over the 12 per-row scalars s[l, :], built with doubling steps on the
vector engine.  The table is laid out rotated by 2048 so the final
output row is the single contiguous slice U[:, 1:4096]:
    U[:, 2048+w] = subset-sum of bits 0..10 over w   (w in [0,2048))
    U[:, c]      = U[:, 2048+c] + s_11               (c in [0,2048))
    => U[:, 1+m] = T[(m + 2049) & 4095] = out[:, m]  (m in [0,4095))
giving one 16380-byte contiguous DMA descriptor per output row.

Output DMAs alternate between the two HWDGE rings per batch, with the
table build deliberately DVE-paced so the rings are almost never
concurrently active: a lone 16-engine DMA stream already saturates the
SBUF AXI ports, two concurrently-active rings make every engine
round-robin between them at packet granularity (+20-35%/descriptor),
and indefinitely-long single-ring runs make SDMA engine 15 degrade
progressively after ~200us.  The table build stays entirely on the
vector engine — scalar-engine SBUF ops contend with DMA reads.

Sharding: data-parallel over the 32 (b,h) pairs, 4 per NeuronCore.
"""

import os
import sys

import numpy as np

if "/opt/trn_rl_repo" not in sys.path:
    sys.path.insert(0, "/opt/trn_rl_repo")

import concourse.bass as bass  # noqa: E402
import concourse.mybir as mybir  # noqa: E402
from concourse import bacc, tile  # noqa: E402
from concourse.bass_utils import run_bass_kernel_spmd  # noqa: E402

F32 = mybir.dt.float32
F16 = mybir.dt.float16

B, H, L, D = 2, 16, 2048, 64
NB = 12                  # bits per position
M = 2 * L - 1            # 4095 relative positions
NCORES = 8
PAIRS = B * H            # 32
PPC = PAIRS // NCORES    # 4 (b,h) pairs per core
ROWS = PPC * L           # 8192 output rows per core


LAST_EXEC_TIME_NS = None


def _build_nc():
    nc = bacc.Bacc(None)
    qT = nc.declare_dram_parameter("qT", [D, ROWS], F32, isOutput=False)
    embT = nc.declare_dram_parameter("embT", [D, NB], F32, isOutput=False)
    out = nc.declare_dram_parameter("out", [ROWS, M], F16, isOutput=True)

    tiles = [(i * 128, 128) for i in range(ROWS // 128)]
    nt = len(tiles)
    NBATCH = 2
    batches = [[i, i + 1] for i in range(0, nt, 2)]

    # input chunks: 8 row-tiles' worth of qT each
    chunks = []
    for g0 in range(0, nt, 8):
        grp = tiles[g0 : g0 + 8]
        c0 = grp[0][0]
        csz = grp[-1][0] + grp[-1][1] - c0
        chunks.append((c0, csz))

    with tile.TileContext(nc) as tc:
        with (
            tc.tile_pool(name="const", bufs=1) as cpool,
            tc.tile_pool(name="psum", bufs=2, space="PSUM") as ppool,
            tc.tile_pool(name="tab", bufs=3) as tpool,
        ):
            embt_sb = cpool.tile([D, NB], F32)
            # s stays f32: tensor_scalar requires a float32 scalar operand
            s_sb = cpool.tile([128, nt * NB], F32)
            qt_chunks = [
                cpool.tile([D, csz], F32, name=f"qt{g}", tag=f"qt{g}")
                for g, (_, csz) in enumerate(chunks)
            ]

            nc.scalar.dma_start(out=embt_sb[:], in_=embT[:])
            for g, (c0, csz) in enumerate(chunks):
                nc.scalar.dma_start(out=qt_chunks[g][:], in_=qT[:, c0 : c0 + csz])

            # s[l, b] = q[l, :] . emb[b, :]; up to 8 row-tiles of s per PSUM bank.
            for g0 in range(0, nt, 8):
                grp = list(range(g0, min(g0 + 8, nt)))
                ps = ppool.tile([128, 8 * NB], F32, name="ps", tag="ps")
                for j, t in enumerate(grp):
                    r0, nr = tiles[t]
                    ci = t // 8
                    off = r0 - chunks[ci][0]
                    nc.tensor.matmul(
                        ps[0:nr, j * NB : (j + 1) * NB],
                        lhsT=qt_chunks[ci][:, off : off + nr],
                        rhs=embt_sb[:],
                        start=True,
                        stop=True,
                    )
                nc.vector.tensor_copy(
                    out=s_sb[:, g0 * NB : (g0 + len(grp)) * NB],
                    in_=ps[:, : len(grp) * NB],
                )

            for batch in batches:
                nr = tiles[batch[0]][1]
                nb = len(batch)
                U = tpool.tile([128, nb * 4096], F16, name="U", tag="U")
                for j, ti in enumerate(batch):
                    sb = ti * NB
                    base = j * 4096
                    hi = base + 2048
                    nc.vector.memset(U[0:nr, hi : hi + 1], 0.0)
                    nc.vector.tensor_copy(
                        out=U[0:nr, hi + 1 : hi + 2], in_=s_sb[0:nr, sb : sb + 1]
                    )
                    for k in range(1, NB - 1):
                        nc.vector.tensor_scalar_add(
                            U[0:nr, hi + 2**k : hi + 2 ** (k + 1)],
                            U[0:nr, hi : hi + 2**k],
                            s_sb[0:nr, sb + k : sb + k + 1],
                        )
                    nc.vector.tensor_scalar_add(
                        U[0:nr, base : base + 2048],
                        U[0:nr, hi : hi + 2048],
                        s_sb[0:nr, sb + NB - 1 : sb + NB],
                    )
                r0 = tiles[batch[0]][0]
                src = U[0:nr].rearrange("p (j c) -> p j c", j=nb)[:, :, 1:4096]
                dst = out[r0 : r0 + nb * nr, :].rearrange("(j p) m -> p j m", p=nr)
                # f16 build runs far ahead of the DMA, so per-batch ring
                # alternation would keep both rings concurrently active
                # (packet-granularity round-robin, +20-35%/descriptor).
                # Instead run each ring for half the batches: overlap at the
                # phase switch is bounded by the tile-pool depth (~2 batches),
                # and each ring stays active well under the ~200us mark where
                # a lone ring starts to degrade.
                eng = nc.sync if batch[0] < tiles[nt // 2][0] else nc.scalar
                eng.dma_start(out=dst, in_=src)

    nc.finalize()
    return nc


def _install_trace_shim():
    """Make run_bass_kernel_spmd(trace=True) work under axon in this
    container: provide antenv.axon_hooks backed by ctypes calls into
    libaxon_pjrt.so, and skip the S3 artifact upload."""
    import contextlib
    import ctypes
    import types

    import antenv
    from concourse import bass_utils

    if getattr(antenv, "axon_hooks", None) is not None:
        return

    def _ntff_profile_via_ctypes(so_path):
        lib = ctypes.CDLL(so_path)
        if not hasattr(lib, "axon_start_nrt_profile"):
            return None
        lib.axon_start_nrt_profile.argtypes = [
            ctypes.POINTER(ctypes.c_int64),
            ctypes.c_size_t,
        ]
        lib.axon_start_nrt_profile.restype = ctypes.c_int64
        lib.axon_stop_nrt_profile.argtypes = [ctypes.c_char_p]
        lib.axon_stop_nrt_profile.restype = ctypes.c_int64

        @contextlib.contextmanager
        def _hook(output_dir, device_ids):
            import jax

            jax.devices()
            if device_ids:
                ids = (ctypes.c_int64 * len(device_ids))(*device_ids)
                rc = lib.axon_start_nrt_profile(ids, len(device_ids))
            else:
                rc = lib.axon_start_nrt_profile(None, 0)
            if rc != 0:
                raise RuntimeError(f"axon_start_nrt_profile rc={rc}")
            try:
                yield
            finally:
                n = lib.axon_stop_nrt_profile(str(output_dir).encode())
                print(f"trace shim: {n} ntff file(s) in {output_dir}", file=sys.stderr)

        return _hook

    mod = types.ModuleType("antenv.axon_hooks")
    state = {"hook": _ntff_profile_via_ctypes("/opt/axon/libaxon_pjrt.so")}
    mod.set_axon_ntff_profile_hook = lambda h: state.__setitem__("hook", h)
    mod.get_axon_ntff_profile_hook = lambda: state["hook"]
    sys.modules["antenv.axon_hooks"] = mod
    antenv.axon_hooks = mod
    bass_utils.upload_artifacts = lambda tmpdir: f"local://{tmpdir}"


def kernel(q, k, emb):
    global LAST_EXEC_TIME_NS
    trace = os.environ.get("KERNEL_TRACE", "") == "1"
    if trace:
        _install_trace_shim()

    nc = _build_nc()

    qr = np.asarray(q, dtype=np.float32).reshape(PAIRS, L, D)
    embT = np.ascontiguousarray(np.asarray(emb, dtype=np.float32).T)
    in_maps = []
    for c in range(NCORES):
        qc = qr[c * PPC : (c + 1) * PPC]  # [PPC, L, D]
        qTc = np.ascontiguousarray(qc.transpose(2, 0, 1).reshape(D, ROWS))
        in_maps.append({"qT": qTc, "embT": embT})

    res = run_bass_kernel_spmd(nc, in_maps, core_ids=list(range(NCORES)), trace=trace)
    LAST_EXEC_TIME_NS = res.exec_time_ns

    out = np.empty((PAIRS, L, M), np.float32)
    for c in range(NCORES):
        # device emits f16 (rel err ~1e-3, gate is 2e-2); widen on gather
        out[c * PPC : (c + 1) * PPC] = res.results[c]["out"].reshape(PPC, L, M)
    return out.reshape(B, H, L, M)



# revision 11
# speedup vs baseline: 1.1288x; 1.1288x over previous
"""Trainium2 Bass kernel for BinaryRelativePositionEmbedding.

Math: out[b,h,l,m] = q[b,h,l,:] . rp[m,:],  rp = bits @ emb, where
bits[m,:] are the 12 two's-complement bits of position (m - L + 1).

Key identity: out[l, m] = sum_b bits[m,b] * s[l,b] with s = q @ emb^T
(rank 12).  The pattern v(m) = (m - (L-1)) & 4095 ranges over all 12-bit
values except 2048, so each row-tile of the output is a subset-sum table
over the 12 per-row scalars s[l, :].  The table is laid out rotated by
2048 so the final output row is the single contiguous slice U[:, 1:4096]:
    U[:, 2048+w] = subset-sum of bits 0..10 over w   (w in [0,2048))
    U[:, c]      = U[:, 2048+c] + s_11               (c in [0,2048))
    => U[:, 1+m] = T[(m + 2049) & 4095] = out[:, m]  (m in [0,4095))

The output is emitted in f16 (the gate is rel_err < 2e-2; f16 build
lands ~1e-3) which halves HBM write traffic -- the binding roofline at
~358 GB/s/core.  The table build is split PE/DVE so the producer runs
~2x faster than the DMA drain and the write stream never starves:

  - bits 0..8 (the 512-wide "stub" of each table) come from the tensor
    engine: stub = q_tile @ R with R = embT[:,0:9] @ bits9, a [64,512]
    constant built on-device by one matmul.  The stub matmul reuses the
    q-tile weights already loaded for the s matmul, so PE cost per tile
    is one weight load + 515 columns.
  - DVE does one PSUM->SBUF f16 copy (512 cols) and three wide
    tensor_scalar adds (512/1024/2048 cols, 2x-mode f16) per table,
    ~1.8us/table vs ~5.9us/table DMA drain.

Output DMAs run one HWDGE ring for the first half of the batches and
the other for the second half: a lone 16-engine stream saturates the
SBUF AXI ports, concurrently-active rings round-robin at packet
granularity (+20-35 %/descriptor), and a single ring degrades past
~200us; each phase here stays under 100us and the producer's lead
bounds ring overlap at the switch to the tile-pool depth.

Sharding: data-parallel over the 32 (b,h) pairs, 4 per NeuronCore.
"""

import os
import sys

import numpy as np

if "/opt/trn_rl_repo" not in sys.path:
    sys.path.insert(0, "/opt/trn_rl_repo")

import concourse.bass as bass  # noqa: E402
import concourse.mybir as mybir  # noqa: E402
from concourse import bacc, tile  # noqa: E402
from concourse.bass_utils import run_bass_kernel_spmd  # noqa: E402

F32 = mybir.dt.float32
F16 = mybir.dt.float16

B, H, L, D = 2, 16, 2048, 64
NB = 12                  # bits per position
SB = 9                   # bits folded into the PE-built stub
SW = 1 << SB             # stub width (512)
M = 2 * L - 1            # 4095 relative positions
NCORES = 8
PAIRS = B * H            # 32
PPC = PAIRS // NCORES    # 4 (b,h) pairs per core
ROWS = PPC * L           # 8192 output rows per core


LAST_EXEC_TIME_NS = None


def _build_nc():
    nc = bacc.Bacc(None)
    qT = nc.declare_dram_parameter("qT", [D, ROWS], F16, isOutput=False)
    emb9 = nc.declare_dram_parameter("emb9", [SB, D], F16, isOutput=False)
    embt3 = nc.declare_dram_parameter("embt3", [D, NB - SB], F16, isOutput=False)
    bits9 = nc.declare_dram_parameter("bits9", [SB, SW], F16, isOutput=False)
    out = nc.declare_dram_parameter("out", [ROWS, M], F16, isOutput=True)

    tiles = [(i * 128, 128) for i in range(ROWS // 128)]
    nt = len(tiles)
    NBATCH = 2
    batches = [[i, i + 1] for i in range(0, nt, 2)]

    # input chunks: 8 row-tiles' worth of qT each
    chunks = []
    for g0 in range(0, nt, 8):
        grp = tiles[g0 : g0 + 8]
        c0 = grp[0][0]
        csz = grp[-1][0] + grp[-1][1] - c0
        chunks.append((c0, csz))

    with tile.TileContext(nc) as tc:
        with (
            tc.tile_pool(name="const", bufs=1) as cpool,
            tc.tile_pool(name="psum", bufs=2, space="PSUM") as ppool,
            tc.tile_pool(name="stub", bufs=4, space="PSUM") as spool,
            tc.tile_pool(name="tab", bufs=4) as tpool,
        ):
            emb9_sb = cpool.tile([SB, D], F16)
            embt3_sb = cpool.tile([D, NB - SB], F16)
            bits9_sb = cpool.tile([SB, SW], F16)
            r_sb = cpool.tile([D, SW], F16)
            s_sb = cpool.tile([128, nt * (NB - SB)], F32)
            qt_chunks = [
                cpool.tile([D, csz], F16, name=f"qt{g}", tag=f"qt{g}")
                for g, (_, csz) in enumerate(chunks)
            ]

            nc.scalar.dma_start(out=emb9_sb[:], in_=emb9[:])
            nc.scalar.dma_start(out=embt3_sb[:], in_=embt3[:])
            nc.scalar.dma_start(out=bits9_sb[:], in_=bits9[:])
            for g, (c0, csz) in enumerate(chunks):
                nc.scalar.dma_start(out=qt_chunks[g][:], in_=qT[:, c0 : c0 + csz])

            # R[d, w] = sum_{b<9} emb[b, d] * bits9[b, w]; stub = q_tile @ R
            r_ps = ppool.tile([D, SW], F32, name="r_ps", tag="r_ps", bufs=1)
            nc.tensor.matmul(
                r_ps[:], lhsT=emb9_sb[:], rhs=bits9_sb[:], start=True, stop=True
            )
            nc.vector.tensor_copy(out=r_sb[:], in_=r_ps[:])

            # per 8-tile group: s[l, 9:12] matmuls; per tile: stub matmul
            stub_ps = {}
            for g0 in range(0, nt, 8):
                grp = list(range(g0, min(g0 + 8, nt)))
                ps = ppool.tile([128, 8 * (NB - SB)], F32, name="ps", tag="ps")
                for j, t in enumerate(grp):
                    r0, nr = tiles[t]
                    ci = t // 8
                    off = r0 - chunks[ci][0]
                    nc.tensor.matmul(
                        ps[0:nr, j * (NB - SB) : (j + 1) * (NB - SB)],
                        lhsT=qt_chunks[ci][:, off : off + nr],
                        rhs=embt3_sb[:],
                        start=True,
                        stop=True,
                    )
                    sp = spool.tile([128, SW], F32, name=f"stub{t}", tag="stub")
                    nc.tensor.matmul(
                        sp[0:nr, :],
                        lhsT=qt_chunks[ci][:, off : off + nr],
                        rhs=r_sb[:],
                        start=True,
                        stop=True,
                    )
                    stub_ps[t] = sp
                nc.vector.tensor_copy(
                    out=s_sb[:, g0 * (NB - SB) : (g0 + len(grp)) * (NB - SB)],
                    in_=ps[:, : len(grp) * (NB - SB)],
                )

            for batch in batches:
                nr = tiles[batch[0]][1]
                nb = len(batch)
                U = tpool.tile([128, nb * 4096], F16, name="U", tag="U")
                for j, ti in enumerate(batch):
                    sb = ti * (NB - SB)
                    base = j * 4096
                    hi = base + 2048
                    nc.vector.tensor_copy(
                        out=U[0:nr, hi : hi + SW], in_=stub_ps[ti][0:nr, :]
                    )
                    for k in range(SB, NB - 1):
                        nc.vector.tensor_scalar_add(
                            U[0:nr, hi + 2**k : hi + 2 ** (k + 1)],
                            U[0:nr, hi : hi + 2**k],
                            s_sb[0:nr, sb + (k - SB) : sb + (k - SB) + 1],
                        )
                    nc.vector.tensor_scalar_add(
                        U[0:nr, base : base + 2048],
                        U[0:nr, hi : hi + 2048],
                        s_sb[0:nr, sb + (NB - 1 - SB) : sb + (NB - SB)],
                    )
                r0 = tiles[batch[0]][0]
                src = U[0:nr].rearrange("p (j c) -> p j c", j=nb)[:, :, 1:4096]
                dst = out[r0 : r0 + nb * nr, :].rearrange("(j p) m -> p j m", p=nr)
                eng = nc.sync if batch[0] < tiles[nt // 2][0] else nc.scalar
                eng.dma_start(out=dst, in_=src)

    nc.finalize()
    return nc


def _install_trace_shim():
    """Make run_bass_kernel_spmd(trace=True) work under axon in this
    container: provide antenv.axon_hooks backed by ctypes calls into
    libaxon_pjrt.so, and skip the S3 artifact upload."""
    import contextlib
    import ctypes
    import types

    import antenv
    from concourse import bass_utils

    if getattr(antenv, "axon_hooks", None) is not None:
        return

    def _ntff_profile_via_ctypes(so_path):
        lib = ctypes.CDLL(so_path)
        if not hasattr(lib, "axon_start_nrt_profile"):
            return None
        lib.axon_start_nrt_profile.argtypes = [
            ctypes.POINTER(ctypes.c_int64),
            ctypes.c_size_t,
        ]
        lib.axon_start_nrt_profile.restype = ctypes.c_int64
        lib.axon_stop_nrt_profile.argtypes = [ctypes.c_char_p]
        lib.axon_stop_nrt_profile.restype = ctypes.c_int64

        @contextlib.contextmanager
        def _hook(output_dir, device_ids):
            import jax

            jax.devices()
            if device_ids:
                ids = (ctypes.c_int64 * len(device_ids))(*device_ids)
                rc = lib.axon_start_nrt_profile(ids, len(device_ids))
            else:
                rc = lib.axon_start_nrt_profile(None, 0)
            if rc != 0:
                raise RuntimeError(f"axon_start_nrt_profile rc={rc}")
            try:
                yield
            finally:
                n = lib.axon_stop_nrt_profile(str(output_dir).encode())
                print(f"trace shim: {n} ntff file(s) in {output_dir}", file=sys.stderr)

        return _hook

    mod = types.ModuleType("antenv.axon_hooks")
    state = {"hook": _ntff_profile_via_ctypes("/opt/axon/libaxon_pjrt.so")}
    mod.set_axon_ntff_profile_hook = lambda h: state.__setitem__("hook", h)
    mod.get_axon_ntff_profile_hook = lambda: state["hook"]
    sys.modules["antenv.axon_hooks"] = mod
    antenv.axon_hooks = mod
    bass_utils.upload_artifacts = lambda tmpdir: f"local://{tmpdir}"


def kernel(q, k, emb):
    global LAST_EXEC_TIME_NS
    trace = os.environ.get("KERNEL_TRACE", "") == "1"
    if trace:
        _install_trace_shim()

    nc = _build_nc()

    qr = np.asarray(q, dtype=np.float32).reshape(PAIRS, L, D)
    embf = np.asarray(emb, dtype=np.float32)
    emb9_h = np.ascontiguousarray(embf[0:SB]).astype(np.float16)
    embt3_h = np.ascontiguousarray(embf.T[:, SB:NB]).astype(np.float16)
    bits9_h = (
        ((np.arange(SW, dtype=np.int64)[None, :] >> np.arange(SB)[:, None]) & 1)
    ).astype(np.float16)
    bits9_h = np.ascontiguousarray(bits9_h)

    in_maps = []
    for c in range(NCORES):
        qc = qr[c * PPC : (c + 1) * PPC]  # [PPC, L, D]
        qTc = np.ascontiguousarray(
            qc.transpose(2, 0, 1).reshape(D, ROWS).astype(np.float16)
        )
        in_maps.append(
            {"qT": qTc, "emb9": emb9_h, "embt3": embt3_h, "bits9": bits9_h}
        )

    res = run_bass_kernel_spmd(nc, in_maps, core_ids=list(range(NCORES)), trace=trace)
    LAST_EXEC_TIME_NS = res.exec_time_ns

    out = np.empty((PAIRS, L, M), np.float32)
    for c in range(NCORES):
        # device emits f16 (rel err ~1e-3, gate is 2e-2); widen on gather
        out[c * PPC : (c + 1) * PPC] = res.results[c]["out"].reshape(PPC, L, M)
    return out.reshape(B, H, L, M)
